# revision 1
# baseline (speedup 1.0000x reference)
"""Trainium2 Bass kernel for nn_HSIM_27771258536586 (histogram_binning).

score = sum_{b,k} min(p,t)/(p + (p==0)) / (B*BINS) over KDE histograms
p,t of pred/target, 30 gaussian bins on [0,1].

Key algorithmic facts exploited:
 - exp(-0.5*((x-c)/delta)^2) == sqrt(pi)/2 * Derivative_Erf((30x - z_b)/sqrt(2))
   and the final score is invariant to any positive rescale of BOTH
   histograms, so the 2/sqrt(pi) constant never needs correcting.
 - ACT's `accum_out` gives the per-partition running sum of the activation
   output in the same single pass, so one bin costs exactly one ACT
   instruction over the core's data; no separate reduce pass needed.

Sharding: data-parallel over B: core c computes the full histogram pair for
batch c (pred[c] on SBUF partitions 0..63, target[c] on partitions 64..127),
its partial score sum_b min/p / 240, then an AllGather + on-device sum
produces the full scalar on every core.
"""

import math

import numpy as np

import concourse.bass as bass
import concourse.mybir as mybir
import concourse.tile as tile
from concourse import bacc, bass_utils

N_CORES = 8
BINS = 30
PP = 64            # pred partitions (target: 64..127)
FC = 2352          # 3*224*224 / 64
F32 = mybir.dt.float32
SQ2 = math.sqrt(2.0)

_cache = {}


def _build(use_collective: bool = True):
    nc = bacc.Bacc(
        "TRN2", target_bir_lowering=False, debug=False, num_devices=N_CORES
    )
    pred_d = nc.dram_tensor("pred", [PP, FC], F32, kind="ExternalInput")
    targ_d = nc.dram_tensor("target", [PP, FC], F32, kind="ExternalInput")
    out_d = nc.dram_tensor("out", [1, 1], F32, kind="ExternalOutput")

    with tile.TileContext(nc) as tc:
        with (
            tc.tile_pool(name="data", bufs=1) as data_pool,
            tc.tile_pool(name="scratch", bufs=2) as scratch_pool,
            tc.tile_pool(name="small", bufs=1) as small_pool,
            tc.tile_pool(name="psum", bufs=1, space="PSUM") as psum_pool,
            tc.tile_pool(name="dram", bufs=1, space="DRAM") as dram_pool,
        ):
            x = data_pool.tile([128, FC], F32)
            nc.sync.dma_start(x[0:PP, :], pred_d[:])
            nc.sync.dma_start(x[PP:128, :], targ_d[:])

            # tiny activation on a const tile: forces the ACT table load to
            # happen during the input DMA instead of after it
            warm = small_pool.tile([1, 2], F32)
            nc.vector.memset(warm[:], 0.0)
            warm2 = small_pool.tile([1, 2], F32)
            nc.scalar.activation(
                warm2[:], warm[:],
                mybir.ActivationFunctionType.Derivative_Erf,
                bias=0.0, scale=1.0,
            )

            # selector weights: col0 = pred rows, col1 = target rows
            sel = small_pool.tile([128, 2], F32)
            nc.vector.memset(sel[:], 0.0)
            nc.vector.memset(sel[0:PP, 0:1], 1.0)
            nc.vector.memset(sel[PP:128, 1:2], 1.0)
            ones8 = small_pool.tile([128, 1], F32)
            nc.vector.memset(ones8[:], 1.0)

            # per-bin bias values as an SBUF tile (bias APs must be [P,1]).
            # Built by ONE writer chain (iota -> cast -> fused affine) so the
            # 30 ACT passes need a single cross-engine wait instead of one
            # per-pass EventSemaphore (~187ns each on the ACT sequencer).
            bias_i = small_pool.tile([128, BINS], mybir.dt.int32)
            nc.gpsimd.iota(bias_i[:], pattern=[[1, BINS]], base=0,
                           channel_multiplier=0)
            bias_f = small_pool.tile([128, BINS], F32)
            nc.vector.tensor_copy(bias_f[:], bias_i[:])
            bias_t = small_pool.tile([128, BINS], F32)
            nc.vector.tensor_scalar(
                bias_t[:], bias_f[:], float(-1.0 / SQ2), float(-0.5 / SQ2),
                op0=mybir.AluOpType.mult, op1=mybir.AluOpType.add,
            )

            # 30 bins: one ACT pass each; accum_out -> column b of R.
            R = small_pool.tile([128, BINS], F32)
            for b in range(BINS):
                dummy = scratch_pool.tile([128, FC], F32, tag="dummy")
                nc.scalar.activation(
                    dummy[:],
                    x[:],
                    mybir.ActivationFunctionType.Derivative_Erf,
                    bias=bias_t[:, b : b + 1],
                    scale=float(30.0 / SQ2),
                    accum_out=R[:, b : b + 1],
                )

            # partition-sum R separately for pred/target rows via selector MMs
            pt = psum_pool.tile([1, 64], F32)
            nc.tensor.matmul(
                pt[0:1, 0:BINS], sel[:, 0:1], R[:, 0:BINS], start=True, stop=True
            )
            nc.tensor.matmul(
                pt[0:1, 32 : 32 + BINS],
                sel[:, 1:2],
                R[:, 0:BINS],
                start=True,
                stop=True,
            )

            ptc = small_pool.tile([1, 64], F32)
            nc.vector.tensor_copy(ptc[:], pt[:])
            P = ptc[0:1, 0:BINS]
            T = ptc[0:1, 32 : 32 + BINS]

            m = small_pool.tile([1, BINS], F32)
            nc.vector.tensor_tensor(m[:], P, T, op=mybir.AluOpType.min)
            mask = small_pool.tile([1, BINS], F32)
            nc.vector.tensor_scalar(
                mask[:], P, 0.0, None, op0=mybir.AluOpType.is_equal
            )
            pd = small_pool.tile([1, BINS], F32)
            nc.vector.tensor_tensor(pd[:], P, mask[:], op=mybir.AluOpType.add)
            rec = small_pool.tile([1, BINS], F32)
            nc.vector.reciprocal(rec[:], pd[:])
            q = small_pool.tile([1, BINS], F32)
            nc.vector.tensor_tensor(q[:], m[:], rec[:], op=mybir.AluOpType.mult)

            s = small_pool.tile([1, 1], F32)
            nc.vector.reduce_sum(s[:], q[:], axis=mybir.AxisListType.X)
            partial = small_pool.tile([1, 8], F32)
            nc.vector.memset(partial[:], 0.0)
            nc.vector.tensor_scalar(
                partial[0:1, 0:1], s[:], 1.0 / (8.0 * BINS), None,
                op0=mybir.AluOpType.mult,
            )

            if use_collective:
                cin = dram_pool.tile([1, 8], F32)
                cout = dram_pool.tile([8, 8], F32)
                nc.gpsimd.dma_start(cin[:], partial[:])
                nc.gpsimd.collective_compute(
                    "AllGather",
                    mybir.AluOpType.bypass,
                    replica_groups=[list(range(N_CORES))],
                    ins=[cin.opt()],
                    outs=[cout.opt()],
                )
                ag = small_pool.tile([8, 8], F32)
                nc.gpsimd.dma_start(ag[:], cout[:])
                fin = psum_pool.tile([1, 8], F32)
                nc.tensor.matmul(
                    fin[0:1, 0:1], ones8[0:8, 0:1], ag[0:8, 0:1],
                    start=True, stop=True,
                )
                fsb = small_pool.tile([1, 1], F32)
                nc.vector.tensor_copy(fsb[:], fin[0:1, 0:1])
                nc.gpsimd.dma_start(out_d[:], fsb[:])
            else:
                nc.gpsimd.dma_start(out_d[:], partial[0:1, 0:1])

    nc.compile()
    return nc


def _get(use_collective: bool = True):
    key = use_collective
    if key not in _cache:
        _cache[key] = _build(use_collective)
    return _cache[key]


def kernel(pred: np.ndarray, target: np.ndarray, _trace: bool = False):
    nc = _get(use_collective=True)
    pred = np.ascontiguousarray(pred, dtype=np.float32)
    target = np.ascontiguousarray(target, dtype=np.float32)
    in_maps = [
        {
            "pred": pred[c].reshape(PP, FC),
            "target": target[c].reshape(PP, FC),
        }
        for c in range(N_CORES)
    ]
    res = bass_utils.run_bass_kernel_spmd(
        nc, in_maps, core_ids=list(range(N_CORES)), trace=_trace
    )
    out = np.float32(res.results[0]["out"][0, 0])
    if _trace:
        kernel.last_result = res
    return np.asarray(out, dtype=np.float32)


if __name__ == "__main__":
    rng = np.random.default_rng(0)
    p = rng.random((8, 3, 224, 224), dtype=np.float32)
    t = rng.random((8, 3, 224, 224), dtype=np.float32)
    print("score:", kernel(p, t))



# revision 2
# speedup vs baseline: 1.3938x; 1.3938x over previous
"""Trainium2 Bass kernel for nn_HSIM_27771258536586 (histogram_binning).

score = sum_{b,k} min(p,t)/(p + (p==0)) / (B*BINS) over KDE histograms
p,t of pred/target, 30 gaussian bins on [0,1].

Structure of the optimization (vs the 30-pass direct version):
 - The 30 bin Gaussians K(z - z_b) (z = 30x, unit sigma, unit spacing) are a
   heavily oversampled family: K(z - z_b) ~= sum_m A[m,b] * K(z - y_m) for
   M < 30 centers y_m on a wider grid, with A computed once on the host by
   least squares (uniform kernel-approx residual ~12% at M=20).  The SAME
   linear operator is applied to both histograms, so its error cancels to
   second order in the scale-invariant score min(p,t)/p: measured end-to-end
   score error is ~4e-5 over many seeds (gate 2e-2).
 - So the kernel runs only M ACT passes (Derivative_Erf, accum_out), then
   recombines per-partition accumulators with two tiny PE matmuls:
     stage 1: g[m, t] = sum_p R[p, m] * sel[p, t]   (R as matmul weights)
     stage 2: h[b, t] = sum_m A[m, b] * g[m, t]
   A, the per-pass biases, the pred/target selector and a ones column are
   shipped as one small host-constant DRAM input.
 - ACT's exp table is warmed during the input DMA; the final min/mask/
   reciprocal tail runs on 30 partitions (free size 1-2) and an AllGather +
   on-device sum produces the full scalar on every core.

Sharding: data-parallel over B: core c computes the histogram pair for batch
c (pred[c] on SBUF partitions 0..63, target[c] on partitions 64..127).
"""

import math

import numpy as np

import concourse.bass as bass
import concourse.mybir as mybir
import concourse.tile as tile
from concourse import bacc, bass_utils

N_CORES = 8
BINS = 30
PP = 64            # pred partitions (target: 64..127)
FC = 2352          # 3*224*224 / 64
F32 = mybir.dt.float32
SQ2 = math.sqrt(2.0)

M = 20             # number of Gaussian evaluation centers (< BINS)
C_MARGIN = 1.0     # centers span [0.5 - m, 29.5 + m]
NCONST = M + BINS + 3   # consts cols: bias | A | sel_pred | sel_targ | ones

_cache = {}


def _host_consts():
    """Least-squares combine matrix A[M, BINS] and the [128, NCONST] const
    block: bias row, A rows, pred/target selector columns, ones column."""
    centers = np.linspace(0.5 - C_MARGIN, 29.5 + C_MARGIN, M)
    zg = np.linspace(0.0, 30.0, 6001)
    phi = np.exp(-0.5 * (zg[:, None] - centers[None, :]) ** 2)
    tgt = np.exp(-0.5 * (zg[:, None] - (np.arange(BINS) + 0.5)[None, :]) ** 2)
    A = np.linalg.solve(phi.T @ phi + 1e-8 * np.eye(M), phi.T @ tgt)

    consts = np.zeros((128, NCONST), dtype=np.float32)
    consts[:, 0:M] = (-centers / SQ2)[None, :]          # per-pass ACT bias
    consts[0:M, M : M + BINS] = A.astype(np.float32)    # combine matrix
    consts[0:PP, M + BINS] = 1.0                        # pred selector
    consts[PP:128, M + BINS + 1] = 1.0                  # target selector
    consts[:, M + BINS + 2] = 1.0                       # ones
    return consts


def _build(use_collective: bool = True):
    nc = bacc.Bacc(
        "TRN2", target_bir_lowering=False, debug=False, num_devices=N_CORES
    )
    pred_d = nc.dram_tensor("pred", [PP, FC], F32, kind="ExternalInput")
    targ_d = nc.dram_tensor("target", [PP, FC], F32, kind="ExternalInput")
    const_d = nc.dram_tensor("consts", [128, NCONST], F32, kind="ExternalInput")
    out_d = nc.dram_tensor("out", [1, 1], F32, kind="ExternalOutput")

    with tile.TileContext(nc) as tc:
        with (
            tc.tile_pool(name="data", bufs=1) as data_pool,
            tc.tile_pool(name="scratch", bufs=2) as scratch_pool,
            tc.tile_pool(name="small", bufs=1) as small_pool,
            tc.tile_pool(name="psum", bufs=1, space="PSUM") as psum_pool,
            tc.tile_pool(name="dram", bufs=1, space="DRAM") as dram_pool,
        ):
            cst = small_pool.tile([128, NCONST], F32)
            nc.sync.dma_start(cst[:], const_d[:])

            x = data_pool.tile([128, FC], F32)
            nc.sync.dma_start(x[0:PP, :], pred_d[:])
            nc.sync.dma_start(x[PP:128, :], targ_d[:])

            # tiny activation on a const tile: forces the ACT table load to
            # happen during the input DMA instead of after it
            warm = small_pool.tile([1, 2], F32)
            nc.vector.memset(warm[:], 0.0)
            warm2 = small_pool.tile([1, 2], F32)
            nc.scalar.activation(
                warm2[:], warm[:],
                mybir.ActivationFunctionType.Derivative_Erf,
                bias=0.0, scale=1.0,
            )

            # M centers: one ACT pass each; accum_out -> column m of R.
            R = small_pool.tile([128, M], F32)
            for m in range(M):
                dummy = scratch_pool.tile([128, FC], F32, tag="dummy")
                nc.scalar.activation(
                    dummy[:],
                    x[:],
                    mybir.ActivationFunctionType.Derivative_Erf,
                    bias=cst[:, m : m + 1],
                    scale=float(30.0 / SQ2),
                    accum_out=R[:, m : m + 1],
                )

            # stage 1: g[m, t] = sum_p R[p, m] * sel[p, t]  (R as weights)
            g_ps = psum_pool.tile([M, 2], F32)
            nc.tensor.matmul(
                g_ps[:], R[:, 0:M], cst[:, M + BINS : M + BINS + 2],
                start=True, stop=True,
            )
            g_sb = small_pool.tile([M, 2], F32)
            nc.vector.tensor_copy(g_sb[:], g_ps[:])

            # stage 2: h[b, t] = sum_m A[m, b] * g[m, t]
            h_ps = psum_pool.tile([BINS, 2], F32)
            nc.tensor.matmul(
                h_ps[:], cst[0:M, M : M + BINS], g_sb[:], start=True, stop=True
            )
            h = small_pool.tile([BINS, 2], F32)
            nc.vector.tensor_copy(h[:], h_ps[:])

            P = h[0:BINS, 0:1]
            T = h[0:BINS, 1:2]
            mt = small_pool.tile([BINS, 1], F32)
            nc.vector.tensor_tensor(mt[:], P, T, op=mybir.AluOpType.min)
            mask = small_pool.tile([BINS, 1], F32)
            nc.vector.tensor_scalar(
                mask[:], P, 0.0, None, op0=mybir.AluOpType.is_equal
            )
            pd = small_pool.tile([BINS, 1], F32)
            nc.vector.tensor_tensor(pd[:], P, mask[:], op=mybir.AluOpType.add)
            rec = small_pool.tile([BINS, 1], F32)
            nc.vector.reciprocal(rec[:], pd[:])
            q = small_pool.tile([BINS, 1], F32)
            nc.vector.tensor_tensor(q[:], mt[:], rec[:], op=mybir.AluOpType.mult)

            # partition-sum q over the 30 bins via ones-matmul
            s_ps = psum_pool.tile([1, 1], F32)
            nc.tensor.matmul(
                s_ps[:], q[:], cst[0:BINS, NCONST - 1 : NCONST],
                start=True, stop=True,
            )
            partial = small_pool.tile([1, 8], F32)
            nc.vector.memset(partial[:], 0.0)
            nc.vector.tensor_scalar(
                partial[0:1, 0:1], s_ps[:], 1.0 / (8.0 * BINS), None,
                op0=mybir.AluOpType.mult,
            )

            if use_collective:
                cin = dram_pool.tile([1, 8], F32)
                cout = dram_pool.tile([8, 8], F32)
                nc.gpsimd.dma_start(cin[:], partial[:])
                nc.gpsimd.collective_compute(
                    "AllGather",
                    mybir.AluOpType.bypass,
                    replica_groups=[list(range(N_CORES))],
                    ins=[cin.opt()],
                    outs=[cout.opt()],
                )
                ag = small_pool.tile([8, 8], F32)
                nc.gpsimd.dma_start(ag[:], cout[:])
                fin = psum_pool.tile([1, 8], F32)
                nc.tensor.matmul(
                    fin[0:1, 0:1], ag[0:8, 0:1], cst[0:8, NCONST - 1 : NCONST],
                    start=True, stop=True,
                )
                fsb = small_pool.tile([1, 1], F32)
                nc.vector.tensor_copy(fsb[:], fin[0:1, 0:1])
                nc.gpsimd.dma_start(out_d[:], fsb[:])
            else:
                nc.gpsimd.dma_start(out_d[:], partial[0:1, 0:1])

    nc.compile()
    return nc


def _get(use_collective: bool = True):
    key = use_collective
    if key not in _cache:
        _cache[key] = _build(use_collective)
    return _cache[key]


def kernel(pred: np.ndarray, target: np.ndarray, _trace: bool = False):
    nc = _get(use_collective=True)
    pred = np.ascontiguousarray(pred, dtype=np.float32)
    target = np.ascontiguousarray(target, dtype=np.float32)
    consts = _host_consts()
    in_maps = [
        {
            "pred": pred[c].reshape(PP, FC),
            "target": target[c].reshape(PP, FC),
            "consts": consts,
        }
        for c in range(N_CORES)
    ]
    res = bass_utils.run_bass_kernel_spmd(
        nc, in_maps, core_ids=list(range(N_CORES)), trace=_trace
    )
    out = np.float32(res.results[0]["out"][0, 0])
    if _trace:
        kernel.last_result = res
    return np.asarray(out, dtype=np.float32)


if __name__ == "__main__":
    rng = np.random.default_rng(0)
    p = rng.random((8, 3, 224, 224), dtype=np.float32)
    t = rng.random((8, 3, 224, 224), dtype=np.float32)
    print("score:", kernel(p, t))


# revision 10
# speedup vs baseline: 1.7148x; 1.2303x over previous
"""Trainium2 Bass kernel for nn_HSIM_27771258536586 (histogram_binning).

score = sum_{b,k} min(p,t)/(p + (p==0)) / (B*BINS) over KDE histograms
p,t of pred/target, 30 gaussian bins on [0,1].

Structure of the optimization (vs the 30-pass direct version):
 - The 30 bin Gaussians K(z - z_b) (z = 30x, unit sigma, unit spacing) are a
   heavily oversampled family: K(z - z_b) ~= sum_m A[m,b] * K(z - y_m) for
   M = 16 centers y_m on a wider grid, with A computed once on the host by
   least squares.  The recombined histograms match the exact ones to <0.5%
   relative, and the SAME linear operator is applied to both histograms, so
   its error cancels further in the scale-invariant score min(p,t)/p:
   measured end-to-end score error is ~6e-5 across seeds (gate 2e-2).
 - So the kernel runs only M ACT passes (Derivative_Erf, accum_out), then
   recombines the per-partition accumulators R[128, M] with two tiny PE
   matmuls:
     stage 1: g[m, t] = sum_p R[p, m] * sel[p, t]   (R as matmul weights)
     stage 2: h[t, b] = sum_m g[m, t] * A[m, b]     (g as matmul weights)
   A, the per-pass biases, the pred/target selector and a ones column are
   shipped as one small host-constant DRAM input.
 - Input is a single fused [128, FC] bf16 tensor (pred rows 0..63, target
   rows 64..127) so the load is one DMA at half the bytes; the consts DMA
   issues from the DVE queue so it does not serialize with it on the SP
   sequencer.  ACT's exp table is warmed during the input DMA.
 - The tail runs on the [2, 30] layout: min / (p==0)+p / reciprocal, then a
   scalar_tensor_tensor with fused accum_out does q = min/p * (1/240) AND
   its free-dim reduction in one DVE op.  AllGather + on-device sum then
   produces the full scalar on every core.

Sharding: data-parallel over B: core c computes the histogram pair for batch
c (pred[c] on SBUF partitions 0..63, target[c] on partitions 64..127).
"""

import math

import numpy as np

import concourse.bass as bass
import concourse.mybir as mybir
import concourse.tile as tile
from concourse import bacc, bass_utils

N_CORES = 8
BINS = 30
PP = 64            # pred partitions (target: 64..127)
FC = 2352          # 3*224*224 / 64
F32 = mybir.dt.float32
BF16 = mybir.dt.bfloat16
SQ2 = math.sqrt(2.0)

M = 16             # number of Gaussian evaluation centers (< BINS)
C_MARGIN = 0.4     # centers span [0.5 - m, 29.5 + m]
NCONST = M + 2 * BINS + 3   # cols: bias | A | sel_pred | sel_targ | ones | A_last

_cache = {}


def _host_consts():
    """Least-squares combine matrix A[M, BINS] and the [128, NCONST] const
    block: bias row, A rows, pred/target selector columns, ones column."""
    centers = np.linspace(0.5 - C_MARGIN, 29.5 + C_MARGIN, M)
    zg = np.linspace(0.0, 30.0, 6001)
    phi = np.exp(-0.5 * (zg[:, None] - centers[None, :]) ** 2)
    tgt = np.exp(-0.5 * (zg[:, None] - (np.arange(BINS) + 0.5)[None, :]) ** 2)
    A = np.linalg.solve(phi.T @ phi + 1e-8 * np.eye(M), phi.T @ tgt)

    consts = np.zeros((128, NCONST), dtype=np.float32)
    consts[:, 0:M] = (-centers / SQ2)[None, :]          # per-pass ACT bias
    consts[0:M, M : M + BINS] = A.astype(np.float32)    # combine matrix
    consts[0:PP, M + BINS] = 1.0                        # pred selector
    consts[PP:128, M + BINS + 1] = 1.0                  # target selector
    consts[:, M + BINS + 2] = 1.0                       # ones
    # last row of A again, on partition 0, for the split stage-2 matmul
    consts[0:1, M + BINS + 3 : M + 2 * BINS + 3] = A[M - 1 : M, :].astype(
        np.float32
    )
    return consts


def _build(use_collective: bool = True):
    nc = bacc.Bacc(
        "TRN2", target_bir_lowering=False, debug=False, num_devices=N_CORES
    )
    x_d = nc.dram_tensor("x", [128, FC], BF16, kind="ExternalInput")
    const_d = nc.dram_tensor("consts", [128, NCONST], F32, kind="ExternalInput")
    out_d = nc.dram_tensor("out", [1, 1], F32, kind="ExternalOutput")

    with tile.TileContext(nc) as tc:
        with (
            tc.tile_pool(name="data", bufs=1) as data_pool,
            tc.tile_pool(name="scratch", bufs=2) as scratch_pool,
            tc.tile_pool(name="small", bufs=1) as small_pool,
            tc.tile_pool(name="psum", bufs=1, space="PSUM") as psum_pool,
            tc.tile_pool(name="dram", bufs=1, space="DRAM") as dram_pool,
        ):
            cst = small_pool.tile([128, NCONST], F32)
            nc.scalar.dma_start(cst[:], const_d[:])

            x = data_pool.tile([128, FC], BF16)
            nc.sync.dma_start(x[:], x_d[:])

            # tiny activation on a const tile: forces the ACT table load to
            # happen during the input DMA instead of after it
            warm = small_pool.tile([1, 2], F32)
            nc.vector.memset(warm[:], 0.0)
            warm2 = small_pool.tile([1, 2], F32)
            nc.scalar.activation(
                warm2[:], warm[:],
                mybir.ActivationFunctionType.Derivative_Erf,
                bias=0.0, scale=1.0,
            )

            # M centers: one ACT pass each; accum_out -> column m of R.
            R = small_pool.tile([128, M], F32)
            for m in range(M):
                dummy = scratch_pool.tile([128, FC], F32, tag="dummy")
                nc.scalar.activation(
                    dummy[:],
                    x[:],
                    mybir.ActivationFunctionType.Derivative_Erf,
                    bias=cst[:, m : m + 1],
                    scale=float(30.0 / SQ2),
                    accum_out=R[:, m : m + 1],
                )

            # stage 1: g[m, t] = sum_p R[p, m] * sel[p, t]  (R as weights).
            # Split so the first M-1 rows of g compute during the last ACT
            # pass; only the final row waits on it.
            sel2 = cst[:, M + BINS : M + BINS + 2]
            g_ps = psum_pool.tile([M - 1, 2], F32)
            nc.tensor.matmul(
                g_ps[:], R[:, 0 : M - 1], sel2, start=True, stop=True
            )
            g2_ps = psum_pool.tile([1, 2], F32)
            nc.tensor.matmul(
                g2_ps[:], R[:, M - 1 : M], sel2, start=True, stop=True
            )
            g_sb = small_pool.tile([M - 1, 2], F32)
            nc.vector.tensor_copy(g_sb[:], g_ps[:])
            g2_sb = small_pool.tile([1, 2], F32)
            nc.vector.tensor_copy(g2_sb[:], g2_ps[:])

            # stage 2: h[b] = sum_m g[m, t] * A[m, b] per tensor t, both
            # landing on partition 0 (pred in psum cols 0:30, target in
            # 32:62) so the whole tail stays on one partition.  Split per
            # tensor and per g-piece: 4 tiny accumulating matmuls.
            A_main = cst[0 : M - 1, M : M + BINS]
            A_last = cst[0:1, M + BINS + 3 : M + 2 * BINS + 3]
            h_ps = psum_pool.tile([1, 64], F32)
            nc.tensor.matmul(
                h_ps[0:1, 0:BINS], g_sb[:, 0:1], A_main,
                start=True, stop=False,
            )
            nc.tensor.matmul(
                h_ps[0:1, 0:BINS], g2_sb[:, 0:1], A_last,
                start=False, stop=True,
            )
            nc.tensor.matmul(
                h_ps[0:1, 32 : 32 + BINS], g_sb[:, 1:2], A_main,
                start=True, stop=False,
            )
            nc.tensor.matmul(
                h_ps[0:1, 32 : 32 + BINS], g2_sb[:, 1:2], A_last,
                start=False, stop=True,
            )
            h = small_pool.tile([1, 64], F32)
            nc.vector.tensor_copy(h[:], h_ps[:])

            P = h[0:1, 0:BINS]
            T = h[0:1, 32 : 32 + BINS]
            mt = small_pool.tile([1, BINS], F32)
            nc.vector.tensor_tensor(mt[:], P, T, op=mybir.AluOpType.min)
            pd = small_pool.tile([1, BINS], F32)
            nc.vector.scalar_tensor_tensor(
                pd[:], P, 0.0, P,
                op0=mybir.AluOpType.is_equal, op1=mybir.AluOpType.add,
            )
            rec = small_pool.tile([1, BINS], F32)
            nc.vector.reciprocal(rec[:], pd[:])

            # q = (min * 1/240) * (1/p), accumulated over bins in the same op
            partial = small_pool.tile([1, 8], F32)
            nc.vector.memset(partial[:], 0.0)
            q = small_pool.tile([1, BINS], F32)
            nc.vector.scalar_tensor_tensor(
                q[:], mt[:], 1.0 / (8.0 * BINS), rec[:],
                op0=mybir.AluOpType.mult, op1=mybir.AluOpType.mult,
                accum_out=partial[0:1, 0:1],
            )

            if use_collective:
                cin = dram_pool.tile([1, 8], F32)
                cout = dram_pool.tile([8, 8], F32)
                nc.gpsimd.dma_start(cin[:], partial[:])
                nc.gpsimd.collective_compute(
                    "AllGather",
                    mybir.AluOpType.bypass,
                    replica_groups=[list(range(N_CORES))],
                    ins=[cin.opt()],
                    outs=[cout.opt()],
                )
                ag = small_pool.tile([8, 8], F32)
                nc.gpsimd.dma_start(ag[:], cout[:])
                fin = psum_pool.tile([1, 8], F32)
                nc.tensor.matmul(
                    fin[0:1, 0:1], ag[0:8, 0:1],
                    cst[0:8, M + BINS + 2 : M + BINS + 3],
                    start=True, stop=True,
                )
                fsb = small_pool.tile([1, 1], F32)
                nc.vector.tensor_copy(fsb[:], fin[0:1, 0:1])
                nc.gpsimd.dma_start(out_d[:], fsb[:])
            else:
                nc.gpsimd.dma_start(out_d[:], partial[0:1, 0:1])

    nc.compile()
    return nc


def _get(use_collective: bool = True):
    key = use_collective
    if key not in _cache:
        _cache[key] = _build(use_collective)
    return _cache[key]


def kernel(pred: np.ndarray, target: np.ndarray, _trace: bool = False):
    import ml_dtypes

    nc = _get(use_collective=True)
    pred = np.ascontiguousarray(pred, dtype=np.float32)
    target = np.ascontiguousarray(target, dtype=np.float32)
    consts = _host_consts()
    in_maps = []
    for c in range(N_CORES):
        xc = np.concatenate(
            [pred[c].reshape(PP, FC), target[c].reshape(PP, FC)], axis=0
        ).astype(ml_dtypes.bfloat16)
        in_maps.append({"x": xc, "consts": consts})
    res = bass_utils.run_bass_kernel_spmd(
        nc, in_maps, core_ids=list(range(N_CORES)), trace=_trace
    )
    out = np.float32(res.results[0]["out"][0, 0])
    if _trace:
        kernel.last_result = res
    return np.asarray(out, dtype=np.float32)


if __name__ == "__main__":
    rng = np.random.default_rng(0)
    p = rng.random((8, 3, 224, 224), dtype=np.float32)
    t = rng.random((8, 3, 224, 224), dtype=np.float32)
    print("score:", kernel(p, t))


# revision 13
# speedup vs baseline: 1.7332x; 1.0107x over previous
"""Trainium2 Bass kernel for nn_HSIM_27771258536586 (histogram_binning).

score = sum_{b,k} min(p,t)/(p + (p==0)) / (B*BINS) over KDE histograms
p,t of pred/target, 30 gaussian bins on [0,1].

Structure of the optimization (vs the 30-pass direct version):
 - The 30 bin Gaussians K(z - z_b) (z = 30x, unit sigma, unit spacing) are a
   heavily oversampled family: K(z - z_b) ~= sum_m A[m,b] * K(z - y_m) for
   M = 16 centers y_m on a wider grid, with A computed once on the host by
   least squares.  The recombined histograms match the exact ones to <0.5%
   relative, and the SAME linear operator is applied to both histograms, so
   its error cancels further in the scale-invariant score min(p,t)/p:
   measured end-to-end score error is ~6e-5 across seeds (gate 2e-2).
 - So the kernel runs only M ACT passes (Derivative_Erf, accum_out), then
   recombines the per-partition accumulators R[128, M] with two tiny PE
   matmuls:
     stage 1: g[m, t] = sum_p R[p, m] * sel[p, t]   (R as matmul weights)
     stage 2: h[t, b] = sum_m g[m, t] * A[m, b]     (g as matmul weights)
   A, the per-pass biases, the pred/target selector and a ones column are
   shipped as one small host-constant DRAM input.
 - Input is a single fused [128, FC] bf16 tensor (pred rows 0..63, target
   rows 64..127) so the load is one DMA at half the bytes; the consts DMA
   issues from the DVE queue so it does not serialize with it on the SP
   sequencer.  ACT's exp table is warmed during the input DMA.
 - The tail runs on the [2, 30] layout: min / (p==0)+p / reciprocal, then a
   scalar_tensor_tensor with fused accum_out does q = min/p * (1/240) AND
   its free-dim reduction in one DVE op.  AllGather + on-device sum then
   produces the full scalar on every core.

Sharding: data-parallel over B: core c computes the histogram pair for batch
c (pred[c] on SBUF partitions 0..63, target[c] on partitions 64..127).
"""

import math

import numpy as np

import concourse.bass as bass
import concourse.mybir as mybir
import concourse.tile as tile
from concourse import bacc, bass_utils

N_CORES = 8
BINS = 30
PP = 64            # pred partitions (target: 64..127)
FC = 2352          # 3*224*224 / 64
F32 = mybir.dt.float32
BF16 = mybir.dt.bfloat16
SQ2 = math.sqrt(2.0)

M = 16             # number of Gaussian evaluation centers (< BINS)
C_MARGIN = 0.4     # centers span [0.5 - m, 29.5 + m]
NCONST = M + 2 * BINS + 3   # cols: bias | A | sel_pred | sel_targ | ones | A_last

_cache = {}


def _host_consts():
    """Least-squares combine matrix A[M, BINS] and the [128, NCONST] const
    block: bias row, A rows, pred/target selector columns, ones column."""
    centers = np.linspace(0.5 - C_MARGIN, 29.5 + C_MARGIN, M)
    zg = np.linspace(0.0, 30.0, 6001)
    phi = np.exp(-0.5 * (zg[:, None] - centers[None, :]) ** 2)
    tgt = np.exp(-0.5 * (zg[:, None] - (np.arange(BINS) + 0.5)[None, :]) ** 2)
    A = np.linalg.solve(phi.T @ phi + 1e-8 * np.eye(M), phi.T @ tgt)

    consts = np.zeros((128, NCONST), dtype=np.float32)
    consts[:, 0:M] = (-centers / SQ2)[None, :]          # per-pass ACT bias
    consts[0:M, M : M + BINS] = A.astype(np.float32)    # combine matrix
    consts[0:PP, M + BINS] = 1.0                        # pred selector
    consts[PP:128, M + BINS + 1] = 1.0                  # target selector
    consts[:, M + BINS + 2] = 1.0                       # ones
    # last row of A again, on partition 0, for the split stage-2 matmul
    consts[0:1, M + BINS + 3 : M + 2 * BINS + 3] = A[M - 1 : M, :].astype(
        np.float32
    )
    return consts


def _build(use_collective: bool = True):
    nc = bacc.Bacc(
        "TRN2", target_bir_lowering=False, debug=False, num_devices=N_CORES
    )
    x_d = nc.dram_tensor("x", [128, FC], BF16, kind="ExternalInput")
    const_d = nc.dram_tensor("consts", [128, NCONST], F32, kind="ExternalInput")
    out_d = nc.dram_tensor("out", [1, 1], F32, kind="ExternalOutput")

    with tile.TileContext(nc) as tc:
        with (
            tc.tile_pool(name="data", bufs=1) as data_pool,
            tc.tile_pool(name="scratch", bufs=2) as scratch_pool,
            tc.tile_pool(name="small", bufs=1) as small_pool,
            tc.tile_pool(name="psum", bufs=1, space="PSUM") as psum_pool,
            tc.tile_pool(name="dram", bufs=1, space="DRAM") as dram_pool,
        ):
            cst = small_pool.tile([128, NCONST], F32)
            nc.scalar.dma_start(cst[:], const_d[:])

            x = data_pool.tile([128, FC], BF16)
            nc.sync.dma_start(x[:], x_d[:])

            # tiny activation on a const tile: forces the ACT table load to
            # happen during the input DMA instead of after it
            warm = small_pool.tile([1, 2], F32)
            nc.vector.memset(warm[:], 0.0)
            warm2 = small_pool.tile([1, 2], F32)
            nc.scalar.activation(
                warm2[:], warm[:],
                mybir.ActivationFunctionType.Derivative_Erf,
                bias=0.0, scale=1.0,
            )

            # M centers: one ACT pass each; accum_out -> column m of R.
            R = small_pool.tile([128, M], F32)
            for m in range(M):
                dummy = scratch_pool.tile([128, FC], F32, tag="dummy")
                nc.scalar.activation(
                    dummy[:],
                    x[:],
                    mybir.ActivationFunctionType.Derivative_Erf,
                    bias=cst[:, m : m + 1],
                    scale=float(30.0 / SQ2),
                    accum_out=R[:, m : m + 1],
                )

            # stage 1: g[m, t] = sum_p R[p, m] * sel[p, t]  (R as weights).
            # Split so the first M-1 rows of g compute during the last ACT
            # pass; only the final row waits on it.
            sel2 = cst[:, M + BINS : M + BINS + 2]
            g_ps = psum_pool.tile([M - 1, 2], F32)
            nc.tensor.matmul(
                g_ps[:], R[:, 0 : M - 1], sel2, start=True, stop=True
            )
            g2_ps = psum_pool.tile([1, 2], F32)
            nc.tensor.matmul(
                g2_ps[:], R[:, M - 1 : M], sel2, start=True, stop=True
            )
            g_sb = small_pool.tile([M - 1, 2], F32)
            nc.vector.tensor_copy(g_sb[:], g_ps[:])
            g2_sb = small_pool.tile([1, 2], F32)
            nc.vector.tensor_copy(g2_sb[:], g2_ps[:])

            # stage 2: h[b] = sum_m g[m, t] * A[m, b] per tensor t, both
            # landing on partition 0 (pred in psum cols 0:30, target in
            # 32:62) so the whole tail stays on one partition.  Split per
            # tensor and per g-piece: 4 tiny accumulating matmuls.
            A_main = cst[0 : M - 1, M : M + BINS]
            A_last = cst[0:1, M + BINS + 3 : M + 2 * BINS + 3]
            h_ps = psum_pool.tile([1, 64], F32)
            nc.tensor.matmul(
                h_ps[0:1, 0:BINS], g_sb[:, 0:1], A_main,
                start=True, stop=False,
            )
            nc.tensor.matmul(
                h_ps[0:1, 0:BINS], g2_sb[:, 0:1], A_last,
                start=False, stop=True,
            )
            nc.tensor.matmul(
                h_ps[0:1, 32 : 32 + BINS], g_sb[:, 1:2], A_main,
                start=True, stop=False,
            )
            nc.tensor.matmul(
                h_ps[0:1, 32 : 32 + BINS], g2_sb[:, 1:2], A_last,
                start=False, stop=True,
            )
            h = small_pool.tile([1, 64], F32)
            nc.vector.tensor_copy(h[:], h_ps[:])
            P = h[0:1, 0:BINS]
            T = h[0:1, 32 : 32 + BINS]
            mt = small_pool.tile([1, BINS], F32)
            nc.vector.tensor_tensor(mt[:], P, T, op=mybir.AluOpType.min)
            pd = small_pool.tile([1, BINS], F32)
            nc.vector.scalar_tensor_tensor(
                pd[:], P, 0.0, P,
                op0=mybir.AluOpType.is_equal, op1=mybir.AluOpType.add,
            )
            rec = small_pool.tile([1, BINS], F32)
            nc.vector.reciprocal(rec[:], pd[:])

            # q = (min * 1/240) * (1/p), accumulated over bins in the same op
            partial = small_pool.tile([1, 8], F32)
            nc.vector.memset(partial[:], 0.0)
            q = small_pool.tile([1, BINS], F32)
            nc.vector.scalar_tensor_tensor(
                q[:], mt[:], 1.0 / (8.0 * BINS), rec[:],
                op0=mybir.AluOpType.mult, op1=mybir.AluOpType.mult,
                accum_out=partial[0:1, 0:1],
            )

            if use_collective:
                cin = dram_pool.tile([1, 8], F32)
                cout = dram_pool.tile([8, 8], F32)
                nc.sync.dma_start(cin[:], partial[:])
                nc.gpsimd.collective_compute(
                    "AllGather",
                    mybir.AluOpType.bypass,
                    replica_groups=[list(range(N_CORES))],
                    ins=[cin.opt()],
                    outs=[cout.opt()],
                )
                ag = small_pool.tile([8, 8], F32)
                nc.sync.dma_start(ag[:], cout[:])
                fin = psum_pool.tile([1, 8], F32)
                nc.tensor.matmul(
                    fin[0:1, 0:1], ag[0:8, 0:1],
                    cst[0:8, M + BINS + 2 : M + BINS + 3],
                    start=True, stop=True,
                )
                fsb = small_pool.tile([1, 1], F32)
                nc.vector.tensor_copy(fsb[:], fin[0:1, 0:1])
                nc.sync.dma_start(out_d[:], fsb[:])
            else:
                nc.sync.dma_start(out_d[:], partial[0:1, 0:1])

    nc.compile()
    return nc


def _get(use_collective: bool = True):
    key = use_collective
    if key not in _cache:
        _cache[key] = _build(use_collective)
    return _cache[key]


def kernel(pred: np.ndarray, target: np.ndarray, _trace: bool = False):
    import ml_dtypes

    nc = _get(use_collective=True)
    pred = np.ascontiguousarray(pred, dtype=np.float32)
    target = np.ascontiguousarray(target, dtype=np.float32)
    consts = _host_consts()
    in_maps = []
    for c in range(N_CORES):
        xc = np.concatenate(
            [pred[c].reshape(PP, FC), target[c].reshape(PP, FC)], axis=0
        ).astype(ml_dtypes.bfloat16)
        in_maps.append({"x": xc, "consts": consts})
    res = bass_utils.run_bass_kernel_spmd(
        nc, in_maps, core_ids=list(range(N_CORES)), trace=_trace
    )
    out = np.float32(res.results[0]["out"][0, 0])
    if _trace:
        kernel.last_result = res
    return np.asarray(out, dtype=np.float32)


if __name__ == "__main__":
    rng = np.random.default_rng(0)
    p = rng.random((8, 3, 224, 224), dtype=np.float32)
    t = rng.random((8, 3, 224, 224), dtype=np.float32)
    print("score:", kernel(p, t))
